# revision 15
# baseline (speedup 1.0000x reference)
"""Multi-head self-attention forward on 8 Trainium2 NeuronCores.

Problem: x[4, 2048, 1024] fp32, weights wq/wk/wv/wo [1024, 1024].
  Q,K,V = x @ w{q,k,v}.T (16 heads x 64); causal softmax(QK^T/8)V; out @ wo.T.

Sharding (single SPMD program, per-core data differs only):
  core c: batch b = c//2, head-half hh = c%2 (heads hh*8..hh*8+8),
  wo-half hh (output dims hh*512..). Per pair (2b, 2b+1):
    - each core: Q/K/V projections for its 8 heads (full 2048 tokens),
      causal flash attention for those heads, producing O^T [512, 2048]
    - pairwise AllGather of O^T -> O_full^T [1024, 2048]
    - each core: out-proj against its 512 output dims -> z [2048, 512]
  Host unshard: out[b][:, hh*512:] = core(2b+hh) output.

v2 pipeline structure (vs the phase-serial v1):
  - All transposes happen on the HOST (numpy): x^T, wq^T, wk^T, wv^T, wo^T
    land in DRAM pre-transposed, so the PE transpose phase is gone.
  - V projection is pipelined against the x^T DMA (per 512-token block).
  - The main loop interleaves, per (head, q-block) unit of attention on
    head-pair c, one sub-unit of the Q^T/K^T projection for chunk c+1.
    The scalar-engine exp stream therefore starts ~40us into the kernel
    and stays busy under PE matmuls instead of serializing after them.
  - S-group software pipeline: S(g+1) is emitted before PV(g) so the PE
    never sits behind a PV that waits on exp(g).
  - GK=2 (2 k-chunks per psum_s group) so QK psum (2 banks) + S psum
    (2x2) + O psum (2x1) fit the 8 PSUM banks exactly.

All matmuls bf16 with fp32 PSUM accumulation; attention in S^T = K@Q^T
orientation so softmax lives on the free axis: exp without max-subtraction
(scores bounded ~|9| for this input distribution), and Vaug = [V_h | ones]
makes PV emit the softmax denominator on psum partitions 64:128 for a
per-partition divide. Per-head K^T/Q^T are written zero-padded to 128
contraction rows directly from PSUM (partition-offset DVE copies) to keep
the PE full-array clock at 2.4 GHz.
"""

import sys

sys.path.insert(0, "/opt/trn_rl_repo")

import ml_dtypes
import numpy as np
import concourse.bass as bass
import concourse.mybir as mybir
import concourse.tile as tile
from concourse import bacc
from concourse.bass_utils import run_bass_kernel_spmd

F32 = mybir.dt.float32
BF16 = mybir.dt.bfloat16
AF = mybir.ActivationFunctionType
OP = mybir.AluOpType

N_CORES = 8
S = 2048          # sequence length
D = 1024          # model dim
HL = 8            # heads per core
DK = 64           # head dim
DL = HL * DK      # local head dims = 512
NEG = -1e30
GK = 2            # k-chunks per psum_s group

_NC_CACHE = {}


def build():
    nc = bacc.Bacc("TRN2", target_bir_lowering=False, debug=False, num_devices=N_CORES)

    # Host-pretransposed inputs.
    # xT:  [128, 4, 8, 512]  partition p, tb, chunk i -> x^T[i*128+p, tb*512+t]
    #      (tb-major so each token-block DMA is one 8KB run per partition)
    # w*T: [128, 8, DL] partition p, chunk i -> w^T[din=i*128+p, dout]
    xT_d = nc.dram_tensor("xT", [128, 4, 8, 512], BF16, kind="ExternalInput")
    wqT_d = nc.dram_tensor("wqT", [128, 8, DL], BF16, kind="ExternalInput")
    wkT_d = nc.dram_tensor("wkT", [128, 8, DL], BF16, kind="ExternalInput")
    wvT_d = nc.dram_tensor("wvT", [128, 8, DL], BF16, kind="ExternalInput")
    woT_d = nc.dram_tensor("woT", [128, 8, DL], BF16, kind="ExternalInput")
    identb_d = nc.dram_tensor("identb", [128, 128], BF16, kind="ExternalInput")
    mtb_d = nc.dram_tensor("mtb", [128, 128], BF16, kind="ExternalInput")
    z = nc.dram_tensor("z", [S, DL], F32, kind="ExternalOutput")

    with tile.TileContext(nc) as tc:
        with (
            tc.tile_pool(name="cst", bufs=1) as cst,
            tc.tile_pool(name="per", bufs=1) as per,       # OT + woT (outlive qkv)
            tc.tile_pool(name="dram", bufs=1, space="DRAM") as dram,
        ):
            # ---------- constants ----------
            identb = cst.tile([128, 128], BF16)
            nc.sync.dma_start(identb[:], identb_d[:])
            mtb = cst.tile([128, 128], BF16)
            nc.sync.dma_start(mtb[:], mtb_d[:])

            # persistent across attn -> out-proj
            OT = per.tile([128, 4, S], BF16)     # [p(dl in chunk), chunk, q]
            woT = per.tile([128, 8, DL], BF16)   # [p(din in chunk), chunk, dout]
            zt1s = []
            for qt in range(16):
                zt1 = per.tile([128, DL], BF16, tag=f"zt1_{qt}")
                zt1s.append(zt1)

            cins = []
            gouts = []
            for j in range(4):
                cin_t = dram.tile([128, S], BF16, tag=f"cin{j}")
                gout_t = dram.tile([256, S], BF16, tag=f"gout{j}")
                cins.append(cin_t)
                gouts.append(gout_t)

            with tc.tile_pool(name="qkv", bufs=1) as qkv:
                xT = qkv.tile([128, 4, 8, 512], BF16)  # [p, tb, i, t]
                wvT = qkv.tile([128, 8, DL], BF16)
                wqT = qkv.tile([128, 8, DL], BF16)
                wkT = qkv.tile([128, 8, DL], BF16)
                # Vaug: [p(tok in kc), h, kc, 0:64]=V, [.., 64:128]=ones
                VA = qkv.tile([128, 8, 16, 128], BF16)
                # padded per-head K^T/Q^T, double-buffered per pair:
                # rows 0:64 live, rows 64:128 zero.
                kq = {}
                for sl in range(2):
                    for nm in ("kA", "qA", "kB", "qB"):
                        kq_t = qkv.tile([128, S], BF16, tag=f"kq{sl}{nm}")
                        kq[(sl, nm)] = kq_t

                # input DMAs (order = need order)
                nc.sync.dma_start(wvT[:], wvT_d[:])
                for tb in range(4):
                    nc.sync.dma_start(xT[:, tb, :, :], xT_d[:, tb, :, :])
                nc.sync.dma_start(wqT[:], wqT_d[:])
                nc.sync.dma_start(wkT[:], wkT_d[:])
                nc.sync.dma_start(woT[:], woT_d[:])

                # ones region of Vaug (gpsimd) and zero rows 64:128 of the
                # padded K/Q tiles (vector) run in parallel at kernel start
                for h in range(HL):
                    nc.gpsimd.memset(VA[:, h, :, DK:128], 1.0)
                for sl in range(2):
                    for nm in ("kA", "qA", "kB", "qB"):
                        nc.vector.memset(kq[(sl, nm)][64:128, :], 0.0)

                with (
                    tc.tile_pool(name="pps", bufs=2, space="PSUM") as pps,
                    tc.tile_pool(name="aps", bufs=2, space="PSUM") as aps,
                    tc.tile_pool(name="apo", bufs=2, space="PSUM") as apo,
                    tc.tile_pool(name="ptp", bufs=3) as ptp,
                    tc.tile_pool(name="dvp", bufs=2) as dvp,
                    tc.tile_pool(name="otfp", bufs=1) as otfp,
                ):
                    otf = {}
                    def stage_otf(j):
                        src, row = (gouts[j], 0) if j < 4 else (gouts[j - 4], 128)
                        ofr = otfp.tile([128, S], BF16, tag=f"otf{j}")
                        nc.sync.dma_start(ofr[:], src[row:row + 128, :])
                        otf[j] = ofr
                    # ---- V projection, pipelined per token-block ----
                    for r in range(16):
                        pp = pps.tile([128, 8, DK], F32, tag="pp")
                        for i in range(8):
                            nc.tensor.matmul(
                                pp[:, :, :],
                                xT[:, r // 4, i, (r % 4) * 128:(r % 4 + 1) * 128],
                                wvT[:, i, :],
                                start=(i == 0), stop=(i == 7))
                        # scatter V into per-head slots of Vaug (one copy)
                        nc.vector.tensor_copy(VA[:, 0:8, r, 0:DK], pp[:, :, :])

                    # ---- QK projection sub-unit emitter ----
                    # chunk c covers heads (2c, 2c+1); unit u in 0..7 emits
                    # one (tensor, tb) psum group + the two padded copies.
                    def qk_unit(c, u):
                        sl = c % 2
                        wT, nms, scale = ((wkT, ("kA", "kB"), None) if u < 4
                                          else (wqT, ("qA", "qB"), 0.125))
                        tb = u % 4
                        pp = pps.tile([128, 512], F32, tag="pp")
                        for i in range(8):
                            nc.tensor.matmul(
                                pp[:],
                                wT[:, i, c * 128:(c + 1) * 128],
                                xT[:, tb, i, :],
                                start=(i == 0), stop=(i == 7))
                        for half, nm in enumerate(nms):
                            dst = kq[(sl, nm)][0:64, tb * 512:(tb + 1) * 512]
                            src = pp[half * 64:(half + 1) * 64, :]
                            if scale is None:
                                nc.vector.tensor_copy(dst, src)
                            else:
                                nc.vector.tensor_scalar_mul(dst, src, scale)

                    # prologue: QK chunk 0
                    for u in range(8):
                        qk_unit(0, u)

                    # ---- attention unit: head h (global), q-block qb ----
                    def attn_unit(h, qb):
                        sl = (h // 2) % 2
                        kth = kq[(sl, "kA" if h % 2 == 0 else "kB")]
                        qth = kq[(sl, "qA" if h % 2 == 0 else "qB")]
                        po = (h % 2) * 64
                        ch = h // 2
                        q0 = qb * 512
                        nkc = 4 * (qb + 1)
                        psum_o = apo.tile([128, 512], F32, tag="po")
                        pend = None  # (pt, offs_kcs) awaiting PV
                        for g0 in range(0, nkc, GK):
                            kcs = list(range(g0, min(g0 + GK, nkc)))
                            psum_s = aps.tile([128, GK * 512], F32, tag="ps")
                            pt = ptp.tile([128, GK * 512], BF16, tag="pt")
                            offs = [i * 512 for i in range(len(kcs))]
                            o = offs[-1] + 512 - max(0, kcs[-1] * 128 - q0)
                            for off, kc in zip(offs, kcs):
                                ws = max(0, kc * 128 - q0)
                                W = 512 - ws
                                diag = kc * 128 >= q0
                                if diag:
                                    nc.tensor.matmul(
                                        psum_s[:, off: off + 128],
                                        identb[:], mtb[:],
                                        start=True, stop=False)
                                nc.tensor.matmul(
                                    psum_s[:, off: off + W],
                                    kth[:, kc * 128:(kc + 1) * 128],
                                    qth[:, q0 + ws: q0 + 512],
                                    start=not diag, stop=True)
                            nc.scalar.activation(
                                pt[:, 0:o], psum_s[:, 0:o], AF.Exp)
                            # PV of the PREVIOUS group (software pipeline)
                            if pend is not None:
                                for off, kc in pend[1]:
                                    ws = max(0, kc * 128 - q0)
                                    nc.tensor.matmul(
                                        psum_o[:, ws:512],
                                        VA[:, h, kc, :],
                                        pend[0][:, off: off + 512 - ws],
                                        start=(kc == 0), stop=(kc == nkc - 1))
                            pend = (pt, list(zip(offs, kcs)))
                        for off, kc in pend[1]:
                            ws = max(0, kc * 128 - q0)
                            nc.tensor.matmul(
                                psum_o[:, ws:512],
                                VA[:, h, kc, :],
                                pend[0][:, off: off + 512 - ws],
                                start=(kc == 0), stop=(kc == nkc - 1))
                        # divide by softmax sum (replicated on rows 64:128)
                        rec = dvp.tile([64, 512], F32, tag="rec")
                        nc.vector.reciprocal(rec[:], psum_o[64:128, :])
                        nc.vector.tensor_tensor(
                            OT[po:po + 64, ch, q0:q0 + 512],
                            psum_o[0:64, :], rec[:], OP.mult)

                    # ---- main loop: pair c attention + chunk c+1 QK ----
                    for c in range(4):
                        units = [(2 * c + hh, qb)
                                 for qb in range(4) for hh in range(2)]
                        for u, (h, qb) in enumerate(units):
                            if c < 3:
                                qk_unit(c + 1, u)
                            attn_unit(h, qb)
                        # chunk complete: exchange it
                        nc.sync.dma_start(cins[c][:], OT[:, c, :])
                        nc.gpsimd.collective_compute(
                            "AllGather", OP.bypass,
                            replica_groups=[[0, 1], [2, 3], [4, 5], [6, 7]],
                            ins=[cins[c][:]], outs=[gouts[c][:]])
                        if c < 3:
                            # stage this chunk's gathered halves for out-proj
                            stage_otf(c)
                            stage_otf(c + 4)

                    # ---- out-proj part 1 (chunks from AllGather 0-2):
                    # runs right as attention drains, hiding AllGather 3
                    g1js = [0, 1, 2, 4, 5, 6]
                    for qt in range(16):
                        pz = pps.tile([128, DL], F32, tag="pp")
                        for n, j in enumerate(g1js):
                            nc.tensor.matmul(
                                pz[:],
                                otf[j][:, qt * 128:(qt + 1) * 128],
                                woT[:, j, :],
                                start=(n == 0), stop=(n == len(g1js) - 1))
                        nc.vector.tensor_copy(zt1s[qt][:], pz[:])

            # ---- out-proj part 2: the last AllGather's chunks + add ----
            with (
                tc.tile_pool(name="otf2", bufs=1) as otf2p,
                tc.tile_pool(name="zsb", bufs=3) as zsb,
                tc.tile_pool(name="zps", bufs=2, space="PSUM") as zps,
            ):
                otf2 = {}
                for j in (3, 7):
                    src, row = (gouts[j], 0) if j < 4 else (gouts[j - 4], 128)
                    ofr = otf2p.tile([128, S], BF16, tag=f"otf{j}")
                    nc.sync.dma_start(ofr[:], src[row:row + 128, :])
                    otf2[j] = ofr
                for qt in range(16):
                    pz = zps.tile([128, DL], F32, tag="pz")
                    for n, j in enumerate((3, 7)):
                        nc.tensor.matmul(
                            pz[:],
                            otf2[j][:, qt * 128:(qt + 1) * 128],
                            woT[:, j, :],
                            start=(n == 0), stop=(n == 1))
                    zt = zsb.tile([128, DL], F32, tag="zt")
                    nc.vector.tensor_tensor(zt[:], pz[:], zt1s[qt][:], OP.add)
                    nc.sync.dma_start(z[qt * 128:(qt + 1) * 128, :], zt[:])

    nc.compile()
    return nc


def _get_nc():
    if "nc" not in _NC_CACHE:
        _NC_CACHE["nc"] = build()
    return _NC_CACHE["nc"]


def kernel(x, wq, wk, wv, wo, _trace=False):
    bf = ml_dtypes.bfloat16
    x = np.asarray(x, dtype=np.float32)
    b, s, d = x.shape
    assert (b, s, d) == (4, S, D)

    def wt(w):
        # w [DL, D] fp32 -> [128, 8, DL] bf16 host-transposed:
        # out[p, i, o] = w[o, i*128+p]
        w = np.asarray(w, dtype=np.float32).astype(bf)
        return np.ascontiguousarray(w.T.reshape(8, 128, DL).transpose(1, 0, 2))

    def xt(xb):
        # xb [S, D] fp32 -> [128, 4, 8, 512] bf16:
        # out[p, tb, i, t] = xb[tb*512+t, i*128+p]
        xb = xb.astype(bf)
        return np.ascontiguousarray(
            xb.T.reshape(8, 128, 4, 512).transpose(1, 2, 0, 3))

    identb = np.eye(128, dtype=np.float32).astype(bf)
    mtb = np.where(np.arange(128)[:, None] > np.arange(128)[None, :],
                   np.float32(NEG), np.float32(0.0)).astype(bf)

    wqs = [wt(np.asarray(wq, np.float32)[hh * DL:(hh + 1) * DL]) for hh in range(2)]
    wks = [wt(np.asarray(wk, np.float32)[hh * DL:(hh + 1) * DL]) for hh in range(2)]
    wvs = [wt(np.asarray(wv, np.float32)[hh * DL:(hh + 1) * DL]) for hh in range(2)]
    wos = [wt(np.asarray(wo, np.float32)[hh * DL:(hh + 1) * DL]) for hh in range(2)]
    xts = [xt(x[bi]) for bi in range(4)]

    in_maps = []
    for c in range(N_CORES):
        bi, hh = c // 2, c % 2
        in_maps.append({
            "xT": xts[bi],
            "wqT": wqs[hh],
            "wkT": wks[hh],
            "wvT": wvs[hh],
            "woT": wos[hh],
            "identb": identb,
            "mtb": mtb,
        })

    nc = _get_nc()
    res = run_bass_kernel_spmd(nc, in_maps, core_ids=list(range(N_CORES)),
                               trace=_trace)

    out = np.empty((4, S, D), dtype=np.float32)
    for c in range(N_CORES):
        bi, hh = c // 2, c % 2
        out[bi][:, hh * DL:(hh + 1) * DL] = res.results[c]["z"]
    if _trace:
        kernel.last_exec_time_ns = res.exec_time_ns
    return out
